# revision 43
# baseline (speedup 1.0000x reference)
"""Multi-head attention (B=4, N=1568, C=768, H=12) on 8 TRN2 NeuronCores.

Sharding: query-parallel. Core c handles batch b = c // 2 and query half
half = c % 2 (784 query tokens). Each core computes K/V projections for the
full 1568 tokens of its batch (duplicated across the pair), Q projection
for its 784 tokens, full attention for all 12 heads over its queries, and
the output projection. No cross-core communication.

Host-side tricks:
  - tokens are rotated per core so its own query half sits at columns 0:784
    of xT; the key order is then a (core-dependent) permutation, which
    softmax attention is invariant to. This removes the separate xqT input.
  - v_bias is folded into the projection bias:
      out = (attn + 1 (x) v_bias) @ proj_w + proj_b
          = attn @ proj_w + (proj_b + v_bias @ proj_w)
  - the softmax 1/sqrt(D) scale is folded into the exp activation's scale.

Schedule (v2): the scalar engine (exp) is the pacer; everything else is
arranged so it never stalls.
  - queries are processed in two blocks of 392 so one head's score tile is
    a single PSUM bank. PSUM budget (8 banks): score-pair ping-pong 2x2 +
    PV accumulator pair 2 + projection-chain scratch 2.
  - heads are processed in pairs (2f, 2f+1). Score matmuls for the pair
    use the two 64-row halves of the PE array concurrently (row tiling via
    base_partition 0/64). One exp activation covers both heads' scores.
  - PV keeps the ones-column denominator trick (M=65, fp32 PSUM accum).
  - K/Q projections for pair f+1 and the V projection (pairs 0-1) are
    interleaved into the attention stream as PE gap filler.
  - output projection is bf16 and split by query block so the first half
    overlaps the last pair's attention.
"""

import numpy as np
import ml_dtypes

B, N, C = 4, 1568, 768
H = 12
D = 64
NQ = N // 2          # 784 queries per core
QB = 392             # query block (2 per core)
SCALE = D ** -0.5
N_CORES = 8
KT = [128] * 12 + [32]          # key tiles (sum = 1568)

_cache = {}


def _build_program():
    import concourse.mybir as mybir
    from concourse import bacc
    from concourse.tile import TileContext

    f32 = mybir.dt.float32
    bf16 = mybir.dt.bfloat16
    Exp = mybir.ActivationFunctionType.Exp

    nc = bacc.Bacc("TRN2", target_bir_lowering=False, debug=False,
                   num_devices=N_CORES)

    xT_d = nc.dram_tensor("xT", [C, N], bf16, kind="ExternalInput")
    # wqk host layout: per feature tile ft, a 256-col block [K-ft | Q-ft]
    wqk_d = nc.dram_tensor("wqk", [C, 2 * C], bf16, kind="ExternalInput")
    wv_d = nc.dram_tensor("wv", [C, C], bf16, kind="ExternalInput")
    wp_d = nc.dram_tensor("wproj", [C, C], bf16, kind="ExternalInput")
    qb_d = nc.dram_tensor("qb", [128, 6], f32, kind="ExternalInput")
    pb_d = nc.dram_tensor("pb", [128, 6], f32, kind="ExternalInput")
    out_d = nc.dram_tensor("outT", [C, NQ], f32, kind="ExternalOutput")

    with TileContext(nc) as tc:
        persist_cm = tc.tile_pool(name="persist", bufs=1)
        persist = persist_cm.__enter__()
        kT = [persist.tile([128, N], bf16, tag=f"kT{j}", name=f"kT{j}")
              for j in range(6)]
        qT = [persist.tile([128, NQ], bf16, tag=f"qT{j}", name=f"qT{j}")
              for j in range(6)]
        v_sb = [persist.tile([128, H * (D + 1)], bf16, tag=f"v{t}", name=f"v{t}")
                for t in range(13)]
        attn = [persist.tile([128, NQ], bf16, tag=f"at{j}", name=f"at{j}")
                for j in range(6)]
        qb_sb = persist.tile([128, 6], f32, tag="qb")
        pb_sb = persist.tile([128, 6], f32, tag="pb")
        wp_sb = [persist.tile([128, C], bf16, tag=f"wp{j}", name=f"wp{j}")
                 for j in range(6)]
        xT = [persist.tile([128, N], bf16, tag=f"xT{j}", name=f"xTs{j}")
              for j in range(6)]
        wqk = [persist.tile([128, 2 * C], bf16, tag=f"wqk{j}", name=f"wqks{j}")
               for j in range(6)]
        wv = [persist.tile([128, C], bf16, tag=f"wv{j}", name=f"wvs{j}")
              for j in range(6)]

        # ---- DMA, issued round-robin over four engine queues (DMA issue
        # costs ~650ns of sequencer time each — serializing 40+ on one
        # queue would dominate startup), ordered so the first head pair
        # can start ASAP.
        # DMA-capable queues: SP, gpsimd, Activation. Scalar only helps
        # before the exp stream begins, so use it for the early transfers.
        early = [nc.sync, nc.gpsimd, nc.scalar]
        late = [nc.sync, nc.gpsimd]
        dma_n = [0]

        def dma(out, in_):
            eng = early if dma_n[0] < 24 else late
            eng[dma_n[0] % len(eng)].dma_start(out=out, in_=in_)
            dma_n[0] += 1

        nc.sync.dma_start(out=qb_sb, in_=qb_d[:])
        nc.sync.dma_start(out=pb_sb, in_=pb_d[:])
        for j in range(6):  # K/Q weights for feature tile 0 (small, first)
            dma(wqk[j][:, 0:256], wqk_d[j * 128:(j + 1) * 128, 0:256])
        # xT is the gating bulk input: first token-halves land first so
        # the K/Q chunk-0/1 chains (tokens 0:784) can start early
        for j in range(6):
            dma(xT[j][:, 0:N // 2], xT_d[j * 128:(j + 1) * 128, 0:N // 2])
        for j in range(6):
            dma(xT[j][:, N // 2:N], xT_d[j * 128:(j + 1) * 128, N // 2:N])
        for j in range(6):
            dma(wv[j], wv_d[j * 128:(j + 1) * 128, :])
        for ft in range(1, 6):
            for j in range(6):
                dma(wqk[j][:, ft * 256:(ft + 1) * 256],
                    wqk_d[j * 128:(j + 1) * 128, ft * 256:(ft + 1) * 256])
        for j in range(6):
            dma(wp_sb[j], wp_d[j * 128:(j + 1) * 128, :])

        # PSUM pools: 4 + 2 + 2 = 8 banks
        psS_cm = tc.tile_pool(name="psS", bufs=2, space="PSUM")
        psS = psS_cm.__enter__()
        psO_cm = tc.tile_pool(name="psO", bufs=1, space="PSUM")
        psO = psO_cm.__enter__()
        psA_cm = tc.tile_pool(name="psA", bufs=2, space="PSUM")
        psA = psA_cm.__enter__()

        phB_cm = tc.tile_pool(name="phB", bufs=36)
        phB = phB_cm.__enter__()
        phBn_cm = tc.tile_pool(name="phBn", bufs=2)
        phBn = phBn_cm.__enter__()
        phC_cm = tc.tile_pool(name="phC", bufs=2)
        phC = phC_cm.__enter__()

        # ---- PE warmup: tiny junk matmuls on already-landed qb_sb keep
        # the HAM activity window busy during the big input DMAs, so the
        # first projection matmuls run at 2.4 GHz instead of 1.2.
        with nc.named_scope("warm"):
            wps = psA.tile([128, 512], f32, tag="psA", name="warm")
            for _ in range(80):
                nc.tensor.matmul(wps[0:6, 0:6], qb_sb, qb_sb,
                                 start=True, stop=True)

        KCH = [(0, 392), (392, 392), (784, 392), (1176, 392)]  # key proj chunks

        def emit_k_chunk(ft, ci):
            # kT/qT gate the NEXT pair's score stream: keep them ahead of
            # the V/PV backlog in every engine queue
            t0, tw = KCH[ci]
            with tc.high_priority(offset=500_000):
                ps = psA.tile([128, 512], f32, tag="psA", name=f"k{ft}_{t0}")
                for j in range(6):
                    nc.tensor.matmul(
                        ps[:, 0:tw],
                        wqk[j][:, ft * 256:ft * 256 + 128],
                        xT[j][:, t0:t0 + tw],
                        start=(j == 0), stop=(j == 5),
                    )
                nc.vector.tensor_copy(kT[ft][:, t0:t0 + tw], ps[:, 0:tw])

        def emit_q_chunk(ft, qb):
            t0 = qb * QB
            with tc.high_priority(offset=500_000):
                ps = psA.tile([128, 512], f32, tag="psA", name=f"q{ft}_{t0}")
                for j in range(6):
                    nc.tensor.matmul(
                        ps[:, 0:QB],
                        wqk[j][:, ft * 256 + 128:ft * 256 + 256],
                        xT[j][:, t0:t0 + QB],
                        start=(j == 0), stop=(j == 5),
                    )
                nc.vector.tensor_scalar(
                    out=qT[ft][:, t0:t0 + QB], in0=ps[:, 0:QB],
                    scalar1=qb_sb[:, ft:ft + 1], scalar2=None,
                    op0=mybir.AluOpType.add,
                )

        def emit_v_quad(vq, tt):
            # V projection for one head quad (heads 4vq..4vq+3) and one
            # key tile — small enough to spread just-in-time across the
            # preceding attention windows
            mt = KT[tt]
            v3 = v_sb[tt].rearrange("p (h e) -> p h e", h=H)
            ps = psA.tile([128, 512], f32, tag="psA", name=f"v{vq}_{tt}")
            for j in range(6):
                nc.tensor.matmul(
                    ps[0:mt, 0:256],
                    xT[j][:, tt * 128:tt * 128 + mt],
                    wv[j][:, vq * 256:(vq + 1) * 256],
                    start=(j == 0), stop=(j == 5),
                )
            nc.vector.tensor_copy(
                v3[0:mt, 4 * vq:4 * vq + 4, 0:64],
                ps[0:mt, 0:256].rearrange("p (h e) -> p h e", h=4),
            )
            if vq == 0:
                nc.vector.memset(v3[0:mt, :, 64:65], 1.0)

        with nc.named_scope("qkv"):
            emit_k_chunk(0, 0)
            emit_q_chunk(0, 0)
            emit_k_chunk(0, 1)
            emit_q_chunk(0, 1)
            emit_k_chunk(0, 2)
            emit_k_chunk(0, 3)

        # per-window PE gap fillers: thunks keyed by (f, qb, tt).
        # V quads are spread just-in-time: quad vq must be complete by
        # window 4*vq (pair 2*vq). K/Q projection chunks for pair f+1 at
        # slots {3,7,11}; their DVE evicts must be traced before the
        # window-end normalize.
        filler = {}
        WIN = [(f, qb) for f in range(6) for qb in range(2)]
        # quad 0: tts 0-3 up front + w0 {7,9,11} + w1 {1,3,5,7,9,11}
        vslots = [(0, s) for s in (7, 9, 11)]
        vslots += [(1, s) for s in (1, 3, 5, 7, 9, 11)]
        # quad 1 (due w4): done by w3 end
        vslots += [(2, s) for s in (7, 8, 9, 10, 11, 12)]
        vslots += [(3, s) for s in (1, 3, 5, 7, 9, 11, 12)]
        # quad 2 (due w8): front-loaded, done by w5 end so the final
        # pairs' PVs run inline and the kernel tail stays short
        vslots += [(4, s) for s in (7, 8, 9, 10, 11, 12)]
        vslots += [(5, s) for s in (1, 3, 5, 7, 9, 11, 12)]
        vwork = [(0, tt) for tt in range(4, 13)]
        vwork += [(1, tt) for tt in range(13)] + [(2, tt) for tt in range(13)]
        assert len(vslots) == len(vwork), (len(vslots), len(vwork))
        for (w, s), (vq, tt) in zip(vslots, vwork):
            filler[WIN[w] + (s,)] = ("v", vq, tt)
        # KQ(f+1) at the FIRST slots of window (f,0): the psA pool
        # rotation assigns slots in trace order, so these must precede
        # any quad chain or they inherit the quads' eviction latency
        for f in range(5):
            filler[(f, 0, 1)] = (emit_k_chunk, f + 1, 0)
            filler[(f, 0, 2)] = (emit_k_chunk, f + 1, 1)
            filler[(f, 0, 3)] = (emit_k_chunk, f + 1, 2)
            filler[(f, 0, 4)] = (emit_k_chunk, f + 1, 3)
            filler[(f, 0, 5)] = (emit_q_chunk, f + 1, 0)
            filler[(f, 0, 6)] = (emit_q_chunk, f + 1, 1)

        # Normalize is software-pipelined: phase 1 (traced at the window
        # end) evacuates po to SBUF — freeing the PSUM bank — and kicks
        # off the denominator recip; phase 2 (traced at the NEXT window's
        # start) does the broadcast + multiplies, so its latency never
        # blocks the next window's DVE evictions.
        norm_pend = {}

        def normalize_phase1(f, qb, po):
            t65 = [phBn.tile([65, QB], f32, tag=f"t65{hi}",
                             name=f"t65_{f}{qb}{hi}") for hi in range(2)]
            for hi in range(2):
                nc.vector.tensor_copy(t65[hi], po[0:65, hi * 512:hi * 512 + QB])
            rec0 = [phBn.tile([1, QB], f32, tag=f"rc0{hi}",
                              name=f"rc0_{f}{qb}{hi}") for hi in range(2)]
            rec1 = [phBn.tile([1, QB], f32, tag=f"rc1{hi}",
                              name=f"rc1_{f}{qb}{hi}") for hi in range(2)]
            for hi in range(2):
                nc.gpsimd.dma_start(out=rec0[hi], in_=t65[hi][64:65, :])
            for hi in range(2):
                nc.vector.reciprocal_approx_fast(out=rec1[hi], in_=rec0[hi])
            norm_pend[(f, qb)] = (t65, rec1)

        def normalize_phase2(f, qb):
            import contextlib
            prio = (tc.high_priority(offset=300_000) if f >= 4
                    else contextlib.nullcontext())
            with prio:
                _normalize_phase2(f, qb)

        def _normalize_phase2(f, qb):
            t65, rec1 = norm_pend.pop((f, qb))
            q0 = qb * QB
            rb = [phBn.tile([64, QB], f32, tag=f"rb{hi}",
                            name=f"rb_{f}{qb}{hi}") for hi in range(2)]
            for hi in range(2):
                nc.gpsimd.partition_broadcast(rb[hi], rec1[hi])
            # head 0: aligned write; head 1: DVE lanes are partition-
            # aligned, so stage then DMA-shift to partitions 64-127
            nc.vector.tensor_mul(
                attn[f][0:64, q0:q0 + QB], t65[0][0:64, :], rb[0])
            stage = phBn.tile([64, QB], bf16, tag="stage",
                              name=f"st_{f}{qb}")
            nc.vector.tensor_mul(stage, t65[1][0:64, :], rb[1])
            nc.gpsimd.dma_start(out=attn[f][64:128, q0:q0 + QB], in_=stage)

        # PV matmuls may only be traced after their v tile's producer has
        # been traced (Tile dependencies are trace-order RAW), so queue
        # them and flush as V tiles are emitted.
        pv_queue = []
        v_emitted = set()

        def emit_pv(f, qb, tt, po, ex):
            v3 = v_sb[tt].rearrange("p (h e) -> p h e", h=H)
            mt = KT[tt]
            import contextlib
            prio = (tc.high_priority(offset=300_000) if f >= 4
                    else contextlib.nullcontext())
            with prio:
                for hi in range(2):
                    nc.tensor.matmul(
                        po[0:65, hi * 512:hi * 512 + QB],
                        v3[0:mt, 2 * f + hi, :],
                        ex[0:mt, hi * QB:(hi + 1) * QB],
                        start=(tt == 0), stop=(tt == 12),
                    )
                if tt == 12:
                    normalize_phase1(f, qb, po)

        def flush_pvs():
            while pv_queue and (pv_queue[0][0] // 2, pv_queue[0][2]) in v_emitted:
                emit_pv(*pv_queue.pop(0))

        def emit_v_and_flush(vq, tt):
            emit_v_quad(vq, tt)
            v_emitted.add((vq, tt))
            flush_pvs()

        for tt in range(4):
            emit_v_and_flush(0, tt)

        # ---- output projection (bf16), split by query block. Chains for
        # ot 0-1 start at the pair-5 window (j=0..4 read attn of pairs
        # 0-4, ready long before); only the j=5 rank update waits for the
        # final normalize, shortening the kernel tail.
        oproj_ps = {}

        def oproj_mms(qb, ot, js, ps, c0=0):
            q0 = qb * QB
            for j in js:
                nc.tensor.matmul(
                    ps[:, c0:c0 + QB],
                    wp_sb[j][:, ot * 128:(ot + 1) * 128],
                    attn[j][:, q0:q0 + QB],
                    start=(j == 0), stop=(j == 5),
                )

        def oproj_evict(qb, ot, ps, c0=0):
            q0 = qb * QB
            ob = phC.tile([128, 512], f32, tag="ob")
            nc.vector.tensor_scalar(
                out=ob[:, 0:QB], in0=ps[:, c0:c0 + QB],
                scalar1=pb_sb[:, ot:ot + 1], scalar2=None,
                op0=mybir.AluOpType.add,
            )
            eng = nc.sync if ot % 2 == 0 else nc.gpsimd
            eng.dma_start(
                out=out_d[ot * 128:(ot + 1) * 128, q0:q0 + QB],
                in_=ob[:, 0:QB])

        def oproj_early(qb):
            for ot in (0, 1):
                ps = psA.tile([128, 512], f32, tag="psA", name=f"o{ot}_{qb}")
                oproj_ps[(qb, ot)] = ps
                oproj_mms(qb, ot, range(5), ps)

        def oproj_finish(qb):
            with nc.named_scope("proj"):
                for ot in (0, 1):
                    ps = oproj_ps[(qb, ot)]
                    oproj_mms(qb, ot, [5], ps)
                    oproj_evict(qb, ot, ps)
                if qb == 1:
                    # the last window's scores are done — run the
                    # remaining chains through the freed psS banks, two
                    # per tile, so they pipeline in parallel with psA
                    for pair in ((2, 3), (4, 5)):
                        sp = psS.tile([128, 1024], f32, tag="psS",
                                      name=f"op{pair[0]}")
                        for idx, ot in enumerate(pair):
                            oproj_mms(qb, ot, range(6), sp, c0=idx * 512)
                        for idx, ot in enumerate(pair):
                            oproj_evict(qb, ot, sp, c0=idx * 512)
                else:
                    for ot in (2, 3, 4, 5):
                        ps = psA.tile([128, 512], f32, tag="psA",
                                      name=f"o{ot}_{qb}")
                        oproj_mms(qb, ot, range(6), ps)
                        oproj_evict(qb, ot, ps)

        with nc.named_scope("attn"):
            for f in range(6):
                for qb in range(2):
                    q0 = qb * QB
                    po = psO.tile([128, 1024], f32, tag="psO",
                                  name=f"po{f}_{qb}")
                    for w in list(norm_pend):
                        normalize_phase2(*w)
                    if f == 5:
                        oproj_early(qb)
                    for tt in range(13):
                        mt = KT[tt]
                        k0 = tt * 128
                        # scores + exp are the pacing skeleton: high
                        # priority so the scheduler never queues them
                        # behind PV bursts or projection chains
                        with tc.high_priority(offset=1_000_000):
                            s = psS.tile([128, 1024], f32, tag="psS",
                                         name=f"s{f}_{qb}_{tt}")
                            # scores for the head pair, packed on row halves
                            nc.tensor.matmul(
                                s[0:mt, 0:QB],
                                kT[f][0:64, k0:k0 + mt],
                                qT[f][0:64, q0:q0 + QB],
                                start=True, stop=True,
                            )
                            nc.tensor.matmul(
                                s[0:mt, 512:512 + QB],
                                kT[f][64:128, k0:k0 + mt],
                                qT[f][64:128, q0:q0 + QB],
                                start=True, stop=True,
                            )
                            # one exp for both heads
                            ex = phB.tile([128, 2 * QB], bf16, tag="ex",
                                          name=f"ex{f}_{qb}_{tt}")
                            nc.scalar.activation(
                                out=ex[0:mt, :].rearrange("p (c q) -> p c q", c=2),
                                in_=s[0:mt, :].rearrange("p (c q) -> p c q", c=2)[:, :, 0:QB],
                                func=Exp, scale=SCALE,
                            )
                        pv_queue.append((f, qb, tt, po, ex))
                        flush_pvs()
                        fill = filler.get((f, qb, tt))
                        if fill is not None:
                            if fill[0] == "v":
                                emit_v_and_flush(fill[1], fill[2])
                            else:
                                fill[0](*fill[1:])
                    # normalize phase1 is emitted by the tt==12 PV flush
                    if f == 5:
                        for w in list(norm_pend):
                            normalize_phase2(*w)
                        oproj_finish(qb)

        phC_cm.__exit__(None, None, None)
        phBn_cm.__exit__(None, None, None)
        phB_cm.__exit__(None, None, None)
        psA_cm.__exit__(None, None, None)
        psO_cm.__exit__(None, None, None)
        psS_cm.__exit__(None, None, None)
        persist_cm.__exit__(None, None, None)

    nc.compile()
    return nc


def _get_program():
    if "nc" not in _cache:
        _cache["nc"] = _build_program()
    return _cache["nc"]


def _make_in_maps(x, qkv_w, q_bias, v_bias, proj_w, proj_b):
    # wqk layout: per feature tile ft a 256-col block [K-ft | Q-ft]
    wq = qkv_w[:, 0:C].reshape(C, 6, 128)
    wk = qkv_w[:, C:2 * C].reshape(C, 6, 128)
    wqk = np.concatenate([wk, wq], axis=2).reshape(C, 2 * C)
    wqk = np.ascontiguousarray(wqk)
    wv = np.ascontiguousarray(qkv_w[:, 2 * C:])       # [C, C]
    qb = np.zeros((128, 6), np.float32)
    qb[:, :] = q_bias.reshape(6, 128).T
    pb_eff = proj_b + v_bias @ proj_w                  # fold v_bias into proj
    pb = np.zeros((128, 6), np.float32)
    pb[:, :] = pb_eff.reshape(6, 128).T

    in_maps = []
    for c in range(N_CORES):
        b, half = c // 2, c % 2
        # rotate tokens so this core's query half sits at columns 0:NQ;
        # key order becomes a permutation, which softmax attention is
        # invariant to
        xT = np.ascontiguousarray(
            np.roll(x[b].T, -half * NQ, axis=1)).astype(ml_dtypes.bfloat16)
        in_maps.append({
            "xT": xT, "wqk": wqk.astype(ml_dtypes.bfloat16),
            "wv": wv.astype(ml_dtypes.bfloat16),
            "wproj": proj_w.astype(ml_dtypes.bfloat16),
            "qb": qb, "pb": pb,
        })
    return in_maps


def kernel(x, qkv_w, q_bias, v_bias, proj_w, proj_b):
    from concourse.bass_utils import run_bass_kernel_spmd

    x = np.asarray(x, dtype=np.float32)
    qkv_w = np.asarray(qkv_w, dtype=np.float32)
    q_bias = np.asarray(q_bias, dtype=np.float32)
    v_bias = np.asarray(v_bias, dtype=np.float32)
    proj_w = np.asarray(proj_w, dtype=np.float32)
    proj_b = np.asarray(proj_b, dtype=np.float32)

    nc = _get_program()
    in_maps = _make_in_maps(x, qkv_w, q_bias, v_bias, proj_w, proj_b)
    _cache["in_maps"] = in_maps

    res = run_bass_kernel_spmd(nc, in_maps, list(range(N_CORES)))
    out = np.empty((B, N, C), np.float32)
    for c in range(N_CORES):
        b, half = c // 2, c % 2
        out[b, half * NQ:(half + 1) * NQ, :] = res.results[c]["outT"].T
    return out


# revision 47
# speedup vs baseline: 1.0163x; 1.0163x over previous
"""Multi-head attention (B=4, N=1568, C=768, H=12) on 8 TRN2 NeuronCores.

Sharding: query-parallel. Core c handles batch b = c // 2 and query half
half = c % 2 (784 query tokens). Each core computes K/V projections for the
full 1568 tokens of its batch (duplicated across the pair), Q projection
for its 784 tokens, full attention for all 12 heads over its queries, and
the output projection. No cross-core communication.

Host-side tricks:
  - tokens are rotated per core so its own query half sits at columns 0:784
    of xT; the key order is then a (core-dependent) permutation, which
    softmax attention is invariant to. This removes the separate xqT input.
  - v_bias is folded into the projection bias:
      out = (attn + 1 (x) v_bias) @ proj_w + proj_b
          = attn @ proj_w + (proj_b + v_bias @ proj_w)
  - the softmax 1/sqrt(D) scale is folded into the exp activation's scale.

Schedule (v2): the scalar engine (exp) is the pacer; everything else is
arranged so it never stalls.
  - queries are processed in two blocks of 392 so one head's score tile is
    a single PSUM bank. PSUM budget (8 banks): score-pair ping-pong 2x2 +
    PV accumulator pair 2 + projection-chain scratch 2.
  - heads are processed in pairs (2f, 2f+1). Score matmuls for the pair
    use the two 64-row halves of the PE array concurrently (row tiling via
    base_partition 0/64). One exp activation covers both heads' scores.
  - PV keeps the ones-column denominator trick (M=65, fp32 PSUM accum).
  - K/Q projections for pair f+1 and the V projection (pairs 0-1) are
    interleaved into the attention stream as PE gap filler.
  - output projection is bf16 and split by query block so the first half
    overlaps the last pair's attention.
"""

import numpy as np
import ml_dtypes

B, N, C = 4, 1568, 768
H = 12
D = 64
NQ = N // 2          # 784 queries per core
QB = 392             # query block (2 per core)
SCALE = D ** -0.5
N_CORES = 8
KT = [128] * 12 + [32]          # key tiles (sum = 1568)

_cache = {}


def _build_program():
    import concourse.mybir as mybir
    from concourse import bacc
    from concourse.tile import TileContext

    f32 = mybir.dt.float32
    bf16 = mybir.dt.bfloat16
    Exp = mybir.ActivationFunctionType.Exp

    nc = bacc.Bacc("TRN2", target_bir_lowering=False, debug=False,
                   num_devices=N_CORES)

    xT_d = nc.dram_tensor("xT", [C, N], bf16, kind="ExternalInput")
    # wqk host layout: per feature tile ft, a 256-col block [K-ft | Q-ft]
    wqk_d = nc.dram_tensor("wqk", [C, 2 * C], bf16, kind="ExternalInput")
    wv_d = nc.dram_tensor("wv", [C, C], bf16, kind="ExternalInput")
    wp_d = nc.dram_tensor("wproj", [C, C], bf16, kind="ExternalInput")
    qb_d = nc.dram_tensor("qb", [128, 6], f32, kind="ExternalInput")
    pb_d = nc.dram_tensor("pb", [128, 6], f32, kind="ExternalInput")
    out_d = nc.dram_tensor("outT", [C, NQ], f32, kind="ExternalOutput")

    with TileContext(nc) as tc:
        persist_cm = tc.tile_pool(name="persist", bufs=1)
        persist = persist_cm.__enter__()
        kT = [persist.tile([128, N], bf16, tag=f"kT{j}", name=f"kT{j}")
              for j in range(6)]
        qT = [persist.tile([128, NQ], bf16, tag=f"qT{j}", name=f"qT{j}")
              for j in range(6)]
        v_sb = [persist.tile([128, H * (D + 1)], bf16, tag=f"v{t}", name=f"v{t}")
                for t in range(13)]
        attn = [persist.tile([128, NQ], bf16, tag=f"at{j}", name=f"at{j}")
                for j in range(6)]
        qb_sb = persist.tile([128, 6], f32, tag="qb")
        pb_sb = persist.tile([128, 6], f32, tag="pb")
        wp_sb = [persist.tile([128, C], bf16, tag=f"wp{j}", name=f"wp{j}")
                 for j in range(6)]
        xT = [persist.tile([128, N], bf16, tag=f"xT{j}", name=f"xTs{j}")
              for j in range(6)]
        wqk = [persist.tile([128, 2 * C], bf16, tag=f"wqk{j}", name=f"wqks{j}")
               for j in range(6)]
        wv = [persist.tile([128, C], bf16, tag=f"wv{j}", name=f"wvs{j}")
              for j in range(6)]

        # ---- DMA, issued round-robin over four engine queues (DMA issue
        # costs ~650ns of sequencer time each — serializing 40+ on one
        # queue would dominate startup), ordered so the first head pair
        # can start ASAP.
        # DMA-capable queues: SP, gpsimd, Activation. Scalar only helps
        # before the exp stream begins, so use it for the early transfers.
        early = [nc.sync, nc.gpsimd, nc.scalar]
        late = [nc.sync, nc.gpsimd]
        dma_n = [0]

        def dma(out, in_):
            eng = early if dma_n[0] < 24 else late
            eng[dma_n[0] % len(eng)].dma_start(out=out, in_=in_)
            dma_n[0] += 1

        nc.sync.dma_start(out=qb_sb, in_=qb_d[:])
        nc.sync.dma_start(out=pb_sb, in_=pb_d[:])
        for j in range(6):  # K/Q weights for feature tile 0 (small, first)
            dma(wqk[j][:, 0:256], wqk_d[j * 128:(j + 1) * 128, 0:256])
        # xT is the gating bulk input: first token-halves land first so
        # the K/Q chunk-0/1 chains (tokens 0:784) can start early
        for j in range(6):
            dma(xT[j][:, 0:N // 2], xT_d[j * 128:(j + 1) * 128, 0:N // 2])
        for j in range(6):
            dma(xT[j][:, N // 2:N], xT_d[j * 128:(j + 1) * 128, N // 2:N])
        for j in range(6):
            dma(wv[j], wv_d[j * 128:(j + 1) * 128, :])
        for ft in range(1, 6):
            for j in range(6):
                dma(wqk[j][:, ft * 256:(ft + 1) * 256],
                    wqk_d[j * 128:(j + 1) * 128, ft * 256:(ft + 1) * 256])
        for j in range(6):
            dma(wp_sb[j], wp_d[j * 128:(j + 1) * 128, :])

        # PSUM pools: 4 + 2 + 2 = 8 banks
        psS_cm = tc.tile_pool(name="psS", bufs=2, space="PSUM")
        psS = psS_cm.__enter__()
        psO_cm = tc.tile_pool(name="psO", bufs=1, space="PSUM")
        psO = psO_cm.__enter__()
        psA_cm = tc.tile_pool(name="psA", bufs=2, space="PSUM")
        psA = psA_cm.__enter__()

        phB_cm = tc.tile_pool(name="phB", bufs=36)
        phB = phB_cm.__enter__()
        phBn_cm = tc.tile_pool(name="phBn", bufs=2)
        phBn = phBn_cm.__enter__()
        phC_cm = tc.tile_pool(name="phC", bufs=2)
        phC = phC_cm.__enter__()

        # ---- PE warmup: tiny junk matmuls on already-landed qb_sb keep
        # the HAM activity window busy during the big input DMAs, so the
        # first projection matmuls run at 2.4 GHz instead of 1.2.
        with nc.named_scope("warm"):
            wps = psA.tile([128, 512], f32, tag="psA", name="warm")
            for _ in range(80):
                nc.tensor.matmul(wps[0:6, 0:6], qb_sb, qb_sb,
                                 start=True, stop=True)

        KCH = [(0, 392), (392, 392), (784, 392), (1176, 392)]  # key proj chunks

        def emit_k_chunk(ft, ci):
            # kT/qT gate the NEXT pair's score stream: keep them ahead of
            # the V/PV backlog in every engine queue
            t0, tw = KCH[ci]
            with tc.high_priority(offset=500_000):
                ps = psA.tile([128, 512], f32, tag="psA", name=f"k{ft}_{t0}")
                for j in range(6):
                    nc.tensor.matmul(
                        ps[:, 0:tw],
                        wqk[j][:, ft * 256:ft * 256 + 128],
                        xT[j][:, t0:t0 + tw],
                        start=(j == 0), stop=(j == 5),
                    )
                nc.vector.tensor_copy(kT[ft][:, t0:t0 + tw], ps[:, 0:tw])

        def emit_q_chunk(ft, qb):
            t0 = qb * QB
            with tc.high_priority(offset=500_000):
                ps = psA.tile([128, 512], f32, tag="psA", name=f"q{ft}_{t0}")
                for j in range(6):
                    nc.tensor.matmul(
                        ps[:, 0:QB],
                        wqk[j][:, ft * 256 + 128:ft * 256 + 256],
                        xT[j][:, t0:t0 + QB],
                        start=(j == 0), stop=(j == 5),
                    )
                nc.vector.tensor_scalar(
                    out=qT[ft][:, t0:t0 + QB], in0=ps[:, 0:QB],
                    scalar1=qb_sb[:, ft:ft + 1], scalar2=None,
                    op0=mybir.AluOpType.add,
                )

        def emit_v_quad(vq, tt):
            # V projection for one head quad (heads 4vq..4vq+3) and one
            # key tile — small enough to spread just-in-time across the
            # preceding attention windows
            mt = KT[tt]
            v3 = v_sb[tt].rearrange("p (h e) -> p h e", h=H)
            ps = psA.tile([128, 512], f32, tag="psA", name=f"v{vq}_{tt}")
            for j in range(6):
                nc.tensor.matmul(
                    ps[0:mt, 0:256],
                    xT[j][:, tt * 128:tt * 128 + mt],
                    wv[j][:, vq * 256:(vq + 1) * 256],
                    start=(j == 0), stop=(j == 5),
                )
            nc.vector.tensor_copy(
                v3[0:mt, 4 * vq:4 * vq + 4, 0:64],
                ps[0:mt, 0:256].rearrange("p (h e) -> p h e", h=4),
            )
            if vq == 0:
                nc.vector.memset(v3[0:mt, :, 64:65], 1.0)

        with nc.named_scope("qkv"):
            emit_k_chunk(0, 0)
            emit_q_chunk(0, 0)
            emit_k_chunk(0, 1)
            emit_q_chunk(0, 1)
            emit_k_chunk(0, 2)
            emit_k_chunk(0, 3)

        # per-window PE gap fillers: thunks keyed by (f, qb, tt).
        # V quads are spread just-in-time: quad vq must be complete by
        # window 4*vq (pair 2*vq). K/Q projection chunks for pair f+1 at
        # slots {3,7,11}; their DVE evicts must be traced before the
        # window-end normalize.
        filler = {}
        WIN = [(f, qb) for f in range(6) for qb in range(2)]
        # quad 0: tts 0-3 up front + w0 {7,9,11} + w1 {1,3,5,7,9,11}
        vslots = [(0, s) for s in (7, 9, 11)]
        vslots += [(1, s) for s in (1, 3, 5, 7, 9, 11)]
        # quad 1 (due w4): done by w3 end
        vslots += [(2, s) for s in (7, 8, 9, 10, 11, 12)]
        vslots += [(3, s) for s in (1, 3, 5, 7, 9, 11, 12)]
        # quad 2 (due w8): front-loaded, done by w5 end so the final
        # pairs' PVs run inline and the kernel tail stays short
        vslots += [(4, s) for s in (7, 8, 9, 10, 11, 12)]
        vslots += [(5, s) for s in (1, 3, 5, 7, 9, 11, 12)]
        vwork = [(0, tt) for tt in range(4, 13)]
        vwork += [(1, tt) for tt in range(13)] + [(2, tt) for tt in range(13)]
        assert len(vslots) == len(vwork), (len(vslots), len(vwork))
        for (w, s), (vq, tt) in zip(vslots, vwork):
            filler[WIN[w] + (s,)] = ("v", vq, tt)
        # KQ(f+1) at the FIRST slots of window (f,0): the psA pool
        # rotation assigns slots in trace order, so these must precede
        # any quad chain or they inherit the quads' eviction latency
        for f in range(5):
            filler[(f, 0, 1)] = (emit_k_chunk, f + 1, 0)
            filler[(f, 0, 2)] = (emit_k_chunk, f + 1, 1)
            filler[(f, 0, 3)] = (emit_k_chunk, f + 1, 2)
            filler[(f, 0, 4)] = (emit_k_chunk, f + 1, 3)
            filler[(f, 0, 5)] = (emit_q_chunk, f + 1, 0)
            filler[(f, 0, 6)] = (emit_q_chunk, f + 1, 1)

        # Normalize is software-pipelined: phase 1 (traced at the window
        # end) evacuates po to SBUF — freeing the PSUM bank — and kicks
        # off the denominator recip; phase 2 (traced at the NEXT window's
        # start) does the broadcast + multiplies, so its latency never
        # blocks the next window's DVE evictions.
        norm_pend = {}

        def normalize_phase1(f, qb, po):
            t65 = [phBn.tile([65, QB], f32, tag=f"t65{hi}",
                             name=f"t65_{f}{qb}{hi}") for hi in range(2)]
            for hi in range(2):
                nc.vector.tensor_copy(t65[hi], po[0:65, hi * 512:hi * 512 + QB])
            rec0 = [phBn.tile([1, QB], f32, tag=f"rc0{hi}",
                              name=f"rc0_{f}{qb}{hi}") for hi in range(2)]
            rec1 = [phBn.tile([1, QB], f32, tag=f"rc1{hi}",
                              name=f"rc1_{f}{qb}{hi}") for hi in range(2)]
            for hi in range(2):
                nc.gpsimd.dma_start(out=rec0[hi], in_=t65[hi][64:65, :])
            for hi in range(2):
                nc.vector.reciprocal_approx_fast(out=rec1[hi], in_=rec0[hi])
            norm_pend[(f, qb)] = (t65, rec1)

        def normalize_phase2(f, qb):
            t65, rec1 = norm_pend.pop((f, qb))
            q0 = qb * QB
            rb = [phBn.tile([64, QB], f32, tag=f"rb{hi}",
                            name=f"rb_{f}{qb}{hi}") for hi in range(2)]
            for hi in range(2):
                nc.gpsimd.partition_broadcast(rb[hi], rec1[hi])
            # head 0: aligned write; head 1: DVE lanes are partition-
            # aligned, so stage then DMA-shift to partitions 64-127
            nc.vector.tensor_mul(
                attn[f][0:64, q0:q0 + QB], t65[0][0:64, :], rb[0])
            stage = phBn.tile([64, QB], bf16, tag="stage",
                              name=f"st_{f}{qb}")
            nc.vector.tensor_mul(stage, t65[1][0:64, :], rb[1])
            nc.gpsimd.dma_start(out=attn[f][64:128, q0:q0 + QB], in_=stage)

        # PV matmuls may only be traced after their v tile's producer has
        # been traced (Tile dependencies are trace-order RAW), so queue
        # them and flush as V tiles are emitted.
        pv_queue = []
        v_emitted = set()

        def emit_pv(f, qb, tt, po, ex):
            v3 = v_sb[tt].rearrange("p (h e) -> p h e", h=H)
            mt = KT[tt]
            for hi in range(2):
                nc.tensor.matmul(
                    po[0:65, hi * 512:hi * 512 + QB],
                    v3[0:mt, 2 * f + hi, :],
                    ex[0:mt, hi * QB:(hi + 1) * QB],
                    start=(tt == 0), stop=(tt == 12),
                )
            if tt == 12:
                normalize_phase1(f, qb, po)

        def flush_pvs():
            while pv_queue and (pv_queue[0][0] // 2, pv_queue[0][2]) in v_emitted:
                emit_pv(*pv_queue.pop(0))

        def emit_v_and_flush(vq, tt):
            emit_v_quad(vq, tt)
            v_emitted.add((vq, tt))
            flush_pvs()

        for tt in range(4):
            emit_v_and_flush(0, tt)

        # ---- output projection (bf16), split by query block. Chains for
        # ot 0-1 start at the pair-5 window (j=0..4 read attn of pairs
        # 0-4, ready long before); only the j=5 rank update waits for the
        # final normalize, shortening the kernel tail.
        oproj_ps = {}

        def oproj_mms(qb, ot, js, ps, c0=0):
            q0 = qb * QB
            for j in js:
                nc.tensor.matmul(
                    ps[:, c0:c0 + QB],
                    wp_sb[j][:, ot * 128:(ot + 1) * 128],
                    attn[j][:, q0:q0 + QB],
                    start=(j == 0), stop=(j == 5),
                )

        def oproj_evict(qb, ot, ps, c0=0):
            q0 = qb * QB
            ob = phC.tile([128, 512], f32, tag="ob")
            nc.vector.tensor_scalar(
                out=ob[:, 0:QB], in0=ps[:, c0:c0 + QB],
                scalar1=pb_sb[:, ot:ot + 1], scalar2=None,
                op0=mybir.AluOpType.add,
            )
            nc.sync.dma_start(
                out=out_d[ot * 128:(ot + 1) * 128, q0:q0 + QB],
                in_=ob[:, 0:QB])

        def oproj_early(qb):
            for ot in (0, 1):
                ps = psA.tile([128, 512], f32, tag="psA", name=f"o{ot}_{qb}")
                oproj_ps[(qb, ot)] = ps
                oproj_mms(qb, ot, range(5), ps)

        def oproj_finish(qb):
            with nc.named_scope("proj"):
                for ot in (0, 1):
                    ps = oproj_ps[(qb, ot)]
                    oproj_mms(qb, ot, [5], ps)
                    oproj_evict(qb, ot, ps)
                if qb == 1:
                    # the last window's scores are done — run the
                    # remaining chains through the freed psS banks, two
                    # per tile, so they pipeline in parallel with psA
                    for pair in ((2, 3), (4, 5)):
                        sp = psS.tile([128, 1024], f32, tag="psS",
                                      name=f"op{pair[0]}")
                        for idx, ot in enumerate(pair):
                            oproj_mms(qb, ot, range(6), sp, c0=idx * 512)
                        for idx, ot in enumerate(pair):
                            oproj_evict(qb, ot, sp, c0=idx * 512)
                else:
                    for ot in (2, 3, 4, 5):
                        ps = psA.tile([128, 512], f32, tag="psA",
                                      name=f"o{ot}_{qb}")
                        oproj_mms(qb, ot, range(6), ps)
                        oproj_evict(qb, ot, ps)

        with nc.named_scope("attn"):
            for f in range(6):
                for qb in range(2):
                    q0 = qb * QB
                    po = psO.tile([128, 1024], f32, tag="psO",
                                  name=f"po{f}_{qb}")
                    for w in list(norm_pend):
                        normalize_phase2(*w)
                    if f == 5:
                        oproj_early(qb)
                    for tt in range(13):
                        mt = KT[tt]
                        k0 = tt * 128
                        # scores + exp are the pacing skeleton: high
                        # priority so the scheduler never queues them
                        # behind PV bursts or projection chains
                        with tc.high_priority(offset=1_000_000):
                            s = psS.tile([128, 1024], f32, tag="psS",
                                         name=f"s{f}_{qb}_{tt}")
                            # scores for the head pair, packed on row halves
                            nc.tensor.matmul(
                                s[0:mt, 0:QB],
                                kT[f][0:64, k0:k0 + mt],
                                qT[f][0:64, q0:q0 + QB],
                                start=True, stop=True,
                            )
                            nc.tensor.matmul(
                                s[0:mt, 512:512 + QB],
                                kT[f][64:128, k0:k0 + mt],
                                qT[f][64:128, q0:q0 + QB],
                                start=True, stop=True,
                            )
                            # one exp for both heads
                            ex = phB.tile([128, 2 * QB], bf16, tag="ex",
                                          name=f"ex{f}_{qb}_{tt}")
                            nc.scalar.activation(
                                out=ex[0:mt, :].rearrange("p (c q) -> p c q", c=2),
                                in_=s[0:mt, :].rearrange("p (c q) -> p c q", c=2)[:, :, 0:QB],
                                func=Exp, scale=SCALE,
                            )
                        pv_queue.append((f, qb, tt, po, ex))
                        flush_pvs()
                        fill = filler.get((f, qb, tt))
                        if fill is not None:
                            if fill[0] == "v":
                                emit_v_and_flush(fill[1], fill[2])
                            else:
                                fill[0](*fill[1:])
                    # normalize phase1 is emitted by the tt==12 PV flush
                    if f == 5:
                        for w in list(norm_pend):
                            normalize_phase2(*w)
                        oproj_finish(qb)

        phC_cm.__exit__(None, None, None)
        phBn_cm.__exit__(None, None, None)
        phB_cm.__exit__(None, None, None)
        psA_cm.__exit__(None, None, None)
        psO_cm.__exit__(None, None, None)
        psS_cm.__exit__(None, None, None)
        persist_cm.__exit__(None, None, None)

    nc.compile()
    return nc


def _get_program():
    if "nc" not in _cache:
        _cache["nc"] = _build_program()
    return _cache["nc"]


def _make_in_maps(x, qkv_w, q_bias, v_bias, proj_w, proj_b):
    # wqk layout: per feature tile ft a 256-col block [K-ft | Q-ft]
    wq = qkv_w[:, 0:C].reshape(C, 6, 128)
    wk = qkv_w[:, C:2 * C].reshape(C, 6, 128)
    wqk = np.concatenate([wk, wq], axis=2).reshape(C, 2 * C)
    wqk = np.ascontiguousarray(wqk)
    wv = np.ascontiguousarray(qkv_w[:, 2 * C:])       # [C, C]
    qb = np.zeros((128, 6), np.float32)
    qb[:, :] = q_bias.reshape(6, 128).T
    pb_eff = proj_b + v_bias @ proj_w                  # fold v_bias into proj
    pb = np.zeros((128, 6), np.float32)
    pb[:, :] = pb_eff.reshape(6, 128).T

    in_maps = []
    for c in range(N_CORES):
        b, half = c // 2, c % 2
        # rotate tokens so this core's query half sits at columns 0:NQ;
        # key order becomes a permutation, which softmax attention is
        # invariant to
        xT = np.ascontiguousarray(
            np.roll(x[b].T, -half * NQ, axis=1)).astype(ml_dtypes.bfloat16)
        in_maps.append({
            "xT": xT, "wqk": wqk.astype(ml_dtypes.bfloat16),
            "wv": wv.astype(ml_dtypes.bfloat16),
            "wproj": proj_w.astype(ml_dtypes.bfloat16),
            "qb": qb, "pb": pb,
        })
    return in_maps


def kernel(x, qkv_w, q_bias, v_bias, proj_w, proj_b):
    from concourse.bass_utils import run_bass_kernel_spmd

    x = np.asarray(x, dtype=np.float32)
    qkv_w = np.asarray(qkv_w, dtype=np.float32)
    q_bias = np.asarray(q_bias, dtype=np.float32)
    v_bias = np.asarray(v_bias, dtype=np.float32)
    proj_w = np.asarray(proj_w, dtype=np.float32)
    proj_b = np.asarray(proj_b, dtype=np.float32)

    nc = _get_program()
    in_maps = _make_in_maps(x, qkv_w, q_bias, v_bias, proj_w, proj_b)
    _cache["in_maps"] = in_maps

    res = run_bass_kernel_spmd(nc, in_maps, list(range(N_CORES)))
    out = np.empty((B, N, C), np.float32)
    for c in range(N_CORES):
        b, half = c // 2, c % 2
        out[b, half * NQ:(half + 1) * NQ, :] = res.results[c]["outT"].T
    return out


# revision 48
# speedup vs baseline: 1.0311x; 1.0146x over previous
"""Multi-head attention (B=4, N=1568, C=768, H=12) on 8 TRN2 NeuronCores.

Sharding: query-parallel. Core c handles batch b = c // 2 and query half
half = c % 2 (784 query tokens). Each core computes K/V projections for the
full 1568 tokens of its batch (duplicated across the pair), Q projection
for its 784 tokens, full attention for all 12 heads over its queries, and
the output projection. No cross-core communication.

Host-side tricks:
  - tokens are rotated per core so its own query half sits at columns 0:784
    of xT; the key order is then a (core-dependent) permutation, which
    softmax attention is invariant to. This removes the separate xqT input.
  - v_bias is folded into the projection bias:
      out = (attn + 1 (x) v_bias) @ proj_w + proj_b
          = attn @ proj_w + (proj_b + v_bias @ proj_w)
  - the softmax 1/sqrt(D) scale is folded into the exp activation's scale.

Schedule (v2): the scalar engine (exp) is the pacer; everything else is
arranged so it never stalls.
  - queries are processed in two blocks of 392 so one head's score tile is
    a single PSUM bank. PSUM budget (8 banks): score-pair ping-pong 2x2 +
    PV accumulator pair 2 + projection-chain scratch 2.
  - heads are processed in pairs (2f, 2f+1). Score matmuls for the pair
    use the two 64-row halves of the PE array concurrently (row tiling via
    base_partition 0/64). One exp activation covers both heads' scores.
  - PV keeps the ones-column denominator trick (M=65, fp32 PSUM accum).
  - K/Q projections for pair f+1 and the V projection (pairs 0-1) are
    interleaved into the attention stream as PE gap filler.
  - output projection is bf16 and split by query block so the first half
    overlaps the last pair's attention.
"""

import numpy as np
import ml_dtypes

B, N, C = 4, 1568, 768
H = 12
D = 64
NQ = N // 2          # 784 queries per core
QB = 392             # query block (2 per core)
SCALE = D ** -0.5
N_CORES = 8
KT = [128] * 12 + [32]          # key tiles (sum = 1568)

_cache = {}


def _build_program():
    import concourse.mybir as mybir
    from concourse import bacc
    from concourse.tile import TileContext

    f32 = mybir.dt.float32
    bf16 = mybir.dt.bfloat16
    Exp = mybir.ActivationFunctionType.Exp

    nc = bacc.Bacc("TRN2", target_bir_lowering=False, debug=False,
                   num_devices=N_CORES)

    xT_d = nc.dram_tensor("xT", [C, N], bf16, kind="ExternalInput")
    # wqk host layout: per feature tile ft, a 256-col block [K-ft | Q-ft]
    wqk_d = nc.dram_tensor("wqk", [C, 2 * C], bf16, kind="ExternalInput")
    wv_d = nc.dram_tensor("wv", [C, C], bf16, kind="ExternalInput")
    wp_d = nc.dram_tensor("wproj", [C, C], bf16, kind="ExternalInput")
    qb_d = nc.dram_tensor("qb", [128, 6], f32, kind="ExternalInput")
    pb_d = nc.dram_tensor("pb", [128, 6], f32, kind="ExternalInput")
    out_d = nc.dram_tensor("outT", [C, NQ], f32, kind="ExternalOutput")

    with TileContext(nc) as tc:
        persist_cm = tc.tile_pool(name="persist", bufs=1)
        persist = persist_cm.__enter__()
        kT = [persist.tile([128, N], bf16, tag=f"kT{j}", name=f"kT{j}")
              for j in range(6)]
        qT = [persist.tile([128, NQ], bf16, tag=f"qT{j}", name=f"qT{j}")
              for j in range(6)]
        v_sb = [persist.tile([128, H * (D + 1)], bf16, tag=f"v{t}", name=f"v{t}")
                for t in range(13)]
        attn = [persist.tile([128, NQ], bf16, tag=f"at{j}", name=f"at{j}")
                for j in range(6)]
        qb_sb = persist.tile([128, 6], f32, tag="qb")
        pb_sb = persist.tile([128, 6], f32, tag="pb")
        wp_sb = [persist.tile([128, C], bf16, tag=f"wp{j}", name=f"wp{j}")
                 for j in range(6)]
        xT = [persist.tile([128, N], bf16, tag=f"xT{j}", name=f"xTs{j}")
              for j in range(6)]
        wqk = [persist.tile([128, 2 * C], bf16, tag=f"wqk{j}", name=f"wqks{j}")
               for j in range(6)]
        wv = [persist.tile([128, C], bf16, tag=f"wv{j}", name=f"wvs{j}")
              for j in range(6)]

        # ---- DMA, issued round-robin over four engine queues (DMA issue
        # costs ~650ns of sequencer time each — serializing 40+ on one
        # queue would dominate startup), ordered so the first head pair
        # can start ASAP.
        # DMA-capable queues: SP, gpsimd, Activation. Scalar only helps
        # before the exp stream begins, so use it for the early transfers.
        early = [nc.sync, nc.gpsimd, nc.scalar]
        late = [nc.sync, nc.gpsimd]
        dma_n = [0]

        def dma(out, in_):
            eng = early if dma_n[0] < 18 else late
            eng[dma_n[0] % len(eng)].dma_start(out=out, in_=in_)
            dma_n[0] += 1

        nc.sync.dma_start(out=qb_sb, in_=qb_d[:])
        nc.sync.dma_start(out=pb_sb, in_=pb_d[:])
        for j in range(6):  # K/Q weights for feature tile 0 (small, first)
            dma(wqk[j][:, 0:256], wqk_d[j * 128:(j + 1) * 128, 0:256])
        # xT in quarters, earliest-needed first: the first score window
        # needs only tokens 0:392 (chain subtile deps are address-range
        # based), so the exp stream can start as soon as piece 1 lands
        for p in range(2):
            for j in range(6):
                dma(xT[j][:, p * 392:(p + 1) * 392],
                    xT_d[j * 128:(j + 1) * 128, p * 392:(p + 1) * 392])
        for p in range(2, 4):
            for j in range(6):
                dma(xT[j][:, p * 392:(p + 1) * 392],
                    xT_d[j * 128:(j + 1) * 128, p * 392:(p + 1) * 392])
        for j in range(6):
            dma(wv[j], wv_d[j * 128:(j + 1) * 128, :])
        for ft in range(1, 6):
            for j in range(6):
                dma(wqk[j][:, ft * 256:(ft + 1) * 256],
                    wqk_d[j * 128:(j + 1) * 128, ft * 256:(ft + 1) * 256])
        for j in range(6):
            dma(wp_sb[j], wp_d[j * 128:(j + 1) * 128, :])

        # PSUM pools: 4 + 2 + 2 = 8 banks
        psS_cm = tc.tile_pool(name="psS", bufs=2, space="PSUM")
        psS = psS_cm.__enter__()
        psO_cm = tc.tile_pool(name="psO", bufs=1, space="PSUM")
        psO = psO_cm.__enter__()
        psA_cm = tc.tile_pool(name="psA", bufs=2, space="PSUM")
        psA = psA_cm.__enter__()

        phB_cm = tc.tile_pool(name="phB", bufs=36)
        phB = phB_cm.__enter__()
        phBn_cm = tc.tile_pool(name="phBn", bufs=2)
        phBn = phBn_cm.__enter__()
        phC_cm = tc.tile_pool(name="phC", bufs=2)
        phC = phC_cm.__enter__()

        # ---- PE warmup: tiny junk matmuls on already-landed qb_sb keep
        # the HAM activity window busy during the big input DMAs, so the
        # first projection matmuls run at 2.4 GHz instead of 1.2.
        with nc.named_scope("warm"):
            wps = psA.tile([128, 512], f32, tag="psA", name="warm")
            for _ in range(80):
                nc.tensor.matmul(wps[0:6, 0:6], qb_sb, qb_sb,
                                 start=True, stop=True)

        KCH = [(0, 392), (392, 392), (784, 392), (1176, 392)]  # key proj chunks

        def emit_k_chunk(ft, ci):
            # kT/qT gate the NEXT pair's score stream: keep them ahead of
            # the V/PV backlog in every engine queue
            t0, tw = KCH[ci]
            with tc.high_priority(offset=500_000):
                ps = psA.tile([128, 512], f32, tag="psA", name=f"k{ft}_{t0}")
                for j in range(6):
                    nc.tensor.matmul(
                        ps[:, 0:tw],
                        wqk[j][:, ft * 256:ft * 256 + 128],
                        xT[j][:, t0:t0 + tw],
                        start=(j == 0), stop=(j == 5),
                    )
                nc.vector.tensor_copy(kT[ft][:, t0:t0 + tw], ps[:, 0:tw])

        def emit_q_chunk(ft, qb):
            t0 = qb * QB
            with tc.high_priority(offset=500_000):
                ps = psA.tile([128, 512], f32, tag="psA", name=f"q{ft}_{t0}")
                for j in range(6):
                    nc.tensor.matmul(
                        ps[:, 0:QB],
                        wqk[j][:, ft * 256 + 128:ft * 256 + 256],
                        xT[j][:, t0:t0 + QB],
                        start=(j == 0), stop=(j == 5),
                    )
                nc.vector.tensor_scalar(
                    out=qT[ft][:, t0:t0 + QB], in0=ps[:, 0:QB],
                    scalar1=qb_sb[:, ft:ft + 1], scalar2=None,
                    op0=mybir.AluOpType.add,
                )

        def emit_v_quad(vq, tt):
            # V projection for one head quad (heads 4vq..4vq+3) and one
            # key tile — small enough to spread just-in-time across the
            # preceding attention windows
            mt = KT[tt]
            v3 = v_sb[tt].rearrange("p (h e) -> p h e", h=H)
            ps = psA.tile([128, 512], f32, tag="psA", name=f"v{vq}_{tt}")
            for j in range(6):
                nc.tensor.matmul(
                    ps[0:mt, 0:256],
                    xT[j][:, tt * 128:tt * 128 + mt],
                    wv[j][:, vq * 256:(vq + 1) * 256],
                    start=(j == 0), stop=(j == 5),
                )
            nc.vector.tensor_copy(
                v3[0:mt, 4 * vq:4 * vq + 4, 0:64],
                ps[0:mt, 0:256].rearrange("p (h e) -> p h e", h=4),
            )
            if vq == 0:
                nc.vector.memset(v3[0:mt, :, 64:65], 1.0)

        with nc.named_scope("qkv"):
            emit_k_chunk(0, 0)
            emit_q_chunk(0, 0)
            emit_k_chunk(0, 1)
            emit_q_chunk(0, 1)
            emit_k_chunk(0, 2)
            emit_k_chunk(0, 3)

        # per-window PE gap fillers: thunks keyed by (f, qb, tt).
        # V quads are spread just-in-time: quad vq must be complete by
        # window 4*vq (pair 2*vq). K/Q projection chunks for pair f+1 at
        # slots {3,7,11}; their DVE evicts must be traced before the
        # window-end normalize.
        filler = {}
        WIN = [(f, qb) for f in range(6) for qb in range(2)]
        # quad 0: tts 0-3 up front + w0 {7,9,11} + w1 {1,3,5,7,9,11}
        vslots = [(0, s) for s in (7, 9, 11)]
        vslots += [(1, s) for s in (1, 3, 5, 7, 9, 11)]
        # quad 1 (due w4): done by w3 end
        vslots += [(2, s) for s in (7, 8, 9, 10, 11, 12)]
        vslots += [(3, s) for s in (1, 3, 5, 7, 9, 11, 12)]
        # quad 2 (due w8): front-loaded, done by w5 end so the final
        # pairs' PVs run inline and the kernel tail stays short
        vslots += [(4, s) for s in (7, 8, 9, 10, 11, 12)]
        vslots += [(5, s) for s in (1, 3, 5, 7, 9, 11, 12)]
        vwork = [(0, tt) for tt in range(4, 13)]
        vwork += [(1, tt) for tt in range(13)] + [(2, tt) for tt in range(13)]
        assert len(vslots) == len(vwork), (len(vslots), len(vwork))
        for (w, s), (vq, tt) in zip(vslots, vwork):
            filler[WIN[w] + (s,)] = ("v", vq, tt)
        # KQ(f+1) at the FIRST slots of window (f,0): the psA pool
        # rotation assigns slots in trace order, so these must precede
        # any quad chain or they inherit the quads' eviction latency
        for f in range(5):
            filler[(f, 0, 1)] = (emit_k_chunk, f + 1, 0)
            filler[(f, 0, 2)] = (emit_k_chunk, f + 1, 1)
            filler[(f, 0, 3)] = (emit_k_chunk, f + 1, 2)
            filler[(f, 0, 4)] = (emit_k_chunk, f + 1, 3)
            filler[(f, 0, 5)] = (emit_q_chunk, f + 1, 0)
            filler[(f, 0, 6)] = (emit_q_chunk, f + 1, 1)

        # Normalize is software-pipelined: phase 1 (traced at the window
        # end) evacuates po to SBUF — freeing the PSUM bank — and kicks
        # off the denominator recip; phase 2 (traced at the NEXT window's
        # start) does the broadcast + multiplies, so its latency never
        # blocks the next window's DVE evictions.
        norm_pend = {}

        def normalize_phase1(f, qb, po):
            t65 = [phBn.tile([65, QB], f32, tag=f"t65{hi}",
                             name=f"t65_{f}{qb}{hi}") for hi in range(2)]
            for hi in range(2):
                nc.vector.tensor_copy(t65[hi], po[0:65, hi * 512:hi * 512 + QB])
            rec0 = [phBn.tile([1, QB], f32, tag=f"rc0{hi}",
                              name=f"rc0_{f}{qb}{hi}") for hi in range(2)]
            rec1 = [phBn.tile([1, QB], f32, tag=f"rc1{hi}",
                              name=f"rc1_{f}{qb}{hi}") for hi in range(2)]
            for hi in range(2):
                nc.gpsimd.dma_start(out=rec0[hi], in_=t65[hi][64:65, :])
            for hi in range(2):
                nc.vector.reciprocal_approx_fast(out=rec1[hi], in_=rec0[hi])
            norm_pend[(f, qb)] = (t65, rec1)

        def normalize_phase2(f, qb):
            t65, rec1 = norm_pend.pop((f, qb))
            q0 = qb * QB
            rb = [phBn.tile([64, QB], f32, tag=f"rb{hi}",
                            name=f"rb_{f}{qb}{hi}") for hi in range(2)]
            for hi in range(2):
                nc.gpsimd.partition_broadcast(rb[hi], rec1[hi])
            # head 0: aligned write; head 1: DVE lanes are partition-
            # aligned, so stage then DMA-shift to partitions 64-127
            nc.vector.tensor_mul(
                attn[f][0:64, q0:q0 + QB], t65[0][0:64, :], rb[0])
            stage = phBn.tile([64, QB], bf16, tag="stage",
                              name=f"st_{f}{qb}")
            nc.vector.tensor_mul(stage, t65[1][0:64, :], rb[1])
            nc.gpsimd.dma_start(out=attn[f][64:128, q0:q0 + QB], in_=stage)

        # PV matmuls may only be traced after their v tile's producer has
        # been traced (Tile dependencies are trace-order RAW), so queue
        # them and flush as V tiles are emitted.
        pv_queue = []
        v_emitted = set()

        def emit_pv(f, qb, tt, po, ex):
            v3 = v_sb[tt].rearrange("p (h e) -> p h e", h=H)
            mt = KT[tt]
            for hi in range(2):
                nc.tensor.matmul(
                    po[0:65, hi * 512:hi * 512 + QB],
                    v3[0:mt, 2 * f + hi, :],
                    ex[0:mt, hi * QB:(hi + 1) * QB],
                    start=(tt == 0), stop=(tt == 12),
                )
            if tt == 12:
                normalize_phase1(f, qb, po)

        def flush_pvs():
            while pv_queue and (pv_queue[0][0] // 2, pv_queue[0][2]) in v_emitted:
                emit_pv(*pv_queue.pop(0))

        def emit_v_and_flush(vq, tt):
            emit_v_quad(vq, tt)
            v_emitted.add((vq, tt))
            flush_pvs()

        for tt in range(4):
            emit_v_and_flush(0, tt)

        # ---- output projection (bf16), split by query block. Chains for
        # ot 0-1 start at the pair-5 window (j=0..4 read attn of pairs
        # 0-4, ready long before); only the j=5 rank update waits for the
        # final normalize, shortening the kernel tail.
        oproj_ps = {}

        def oproj_mms(qb, ot, js, ps, c0=0):
            q0 = qb * QB
            for j in js:
                nc.tensor.matmul(
                    ps[:, c0:c0 + QB],
                    wp_sb[j][:, ot * 128:(ot + 1) * 128],
                    attn[j][:, q0:q0 + QB],
                    start=(j == 0), stop=(j == 5),
                )

        def oproj_evict(qb, ot, ps, c0=0):
            q0 = qb * QB
            ob = phC.tile([128, 512], f32, tag="ob")
            nc.vector.tensor_scalar(
                out=ob[:, 0:QB], in0=ps[:, c0:c0 + QB],
                scalar1=pb_sb[:, ot:ot + 1], scalar2=None,
                op0=mybir.AluOpType.add,
            )
            nc.sync.dma_start(
                out=out_d[ot * 128:(ot + 1) * 128, q0:q0 + QB],
                in_=ob[:, 0:QB])

        def oproj_early(qb):
            for ot in (0, 1):
                ps = psA.tile([128, 512], f32, tag="psA", name=f"o{ot}_{qb}")
                oproj_ps[(qb, ot)] = ps
                oproj_mms(qb, ot, range(5), ps)

        def oproj_finish(qb):
            with nc.named_scope("proj"):
                for ot in (0, 1):
                    ps = oproj_ps[(qb, ot)]
                    oproj_mms(qb, ot, [5], ps)
                    oproj_evict(qb, ot, ps)
                if qb == 1:
                    # the last window's scores are done — run the
                    # remaining chains through the freed psS banks, two
                    # per tile, so they pipeline in parallel with psA
                    for pair in ((2, 3), (4, 5)):
                        sp = psS.tile([128, 1024], f32, tag="psS",
                                      name=f"op{pair[0]}")
                        for idx, ot in enumerate(pair):
                            oproj_mms(qb, ot, range(6), sp, c0=idx * 512)
                        for idx, ot in enumerate(pair):
                            oproj_evict(qb, ot, sp, c0=idx * 512)
                else:
                    for ot in (2, 3, 4, 5):
                        ps = psA.tile([128, 512], f32, tag="psA",
                                      name=f"o{ot}_{qb}")
                        oproj_mms(qb, ot, range(6), ps)
                        oproj_evict(qb, ot, ps)

        with nc.named_scope("attn"):
            for f in range(6):
                for qb in range(2):
                    q0 = qb * QB
                    po = psO.tile([128, 1024], f32, tag="psO",
                                  name=f"po{f}_{qb}")
                    for w in list(norm_pend):
                        normalize_phase2(*w)
                    if f == 5:
                        oproj_early(qb)
                    for tt in range(13):
                        mt = KT[tt]
                        k0 = tt * 128
                        # scores + exp are the pacing skeleton: high
                        # priority so the scheduler never queues them
                        # behind PV bursts or projection chains
                        with tc.high_priority(offset=1_000_000):
                            s = psS.tile([128, 1024], f32, tag="psS",
                                         name=f"s{f}_{qb}_{tt}")
                            # scores for the head pair, packed on row halves
                            nc.tensor.matmul(
                                s[0:mt, 0:QB],
                                kT[f][0:64, k0:k0 + mt],
                                qT[f][0:64, q0:q0 + QB],
                                start=True, stop=True,
                            )
                            nc.tensor.matmul(
                                s[0:mt, 512:512 + QB],
                                kT[f][64:128, k0:k0 + mt],
                                qT[f][64:128, q0:q0 + QB],
                                start=True, stop=True,
                            )
                            # one exp for both heads
                            ex = phB.tile([128, 2 * QB], bf16, tag="ex",
                                          name=f"ex{f}_{qb}_{tt}")
                            nc.scalar.activation(
                                out=ex[0:mt, :].rearrange("p (c q) -> p c q", c=2),
                                in_=s[0:mt, :].rearrange("p (c q) -> p c q", c=2)[:, :, 0:QB],
                                func=Exp, scale=SCALE,
                            )
                        pv_queue.append((f, qb, tt, po, ex))
                        flush_pvs()
                        fill = filler.get((f, qb, tt))
                        if fill is not None:
                            if fill[0] == "v":
                                emit_v_and_flush(fill[1], fill[2])
                            else:
                                fill[0](*fill[1:])
                    # normalize phase1 is emitted by the tt==12 PV flush
                    if f == 5:
                        for w in list(norm_pend):
                            normalize_phase2(*w)
                        oproj_finish(qb)

        phC_cm.__exit__(None, None, None)
        phBn_cm.__exit__(None, None, None)
        phB_cm.__exit__(None, None, None)
        psA_cm.__exit__(None, None, None)
        psO_cm.__exit__(None, None, None)
        psS_cm.__exit__(None, None, None)
        persist_cm.__exit__(None, None, None)

    nc.compile()
    return nc


def _get_program():
    if "nc" not in _cache:
        _cache["nc"] = _build_program()
    return _cache["nc"]


def _make_in_maps(x, qkv_w, q_bias, v_bias, proj_w, proj_b):
    # wqk layout: per feature tile ft a 256-col block [K-ft | Q-ft]
    wq = qkv_w[:, 0:C].reshape(C, 6, 128)
    wk = qkv_w[:, C:2 * C].reshape(C, 6, 128)
    wqk = np.concatenate([wk, wq], axis=2).reshape(C, 2 * C)
    wqk = np.ascontiguousarray(wqk)
    wv = np.ascontiguousarray(qkv_w[:, 2 * C:])       # [C, C]
    qb = np.zeros((128, 6), np.float32)
    qb[:, :] = q_bias.reshape(6, 128).T
    pb_eff = proj_b + v_bias @ proj_w                  # fold v_bias into proj
    pb = np.zeros((128, 6), np.float32)
    pb[:, :] = pb_eff.reshape(6, 128).T

    in_maps = []
    for c in range(N_CORES):
        b, half = c // 2, c % 2
        # rotate tokens so this core's query half sits at columns 0:NQ;
        # key order becomes a permutation, which softmax attention is
        # invariant to
        xT = np.ascontiguousarray(
            np.roll(x[b].T, -half * NQ, axis=1)).astype(ml_dtypes.bfloat16)
        in_maps.append({
            "xT": xT, "wqk": wqk.astype(ml_dtypes.bfloat16),
            "wv": wv.astype(ml_dtypes.bfloat16),
            "wproj": proj_w.astype(ml_dtypes.bfloat16),
            "qb": qb, "pb": pb,
        })
    return in_maps


def kernel(x, qkv_w, q_bias, v_bias, proj_w, proj_b):
    from concourse.bass_utils import run_bass_kernel_spmd

    x = np.asarray(x, dtype=np.float32)
    qkv_w = np.asarray(qkv_w, dtype=np.float32)
    q_bias = np.asarray(q_bias, dtype=np.float32)
    v_bias = np.asarray(v_bias, dtype=np.float32)
    proj_w = np.asarray(proj_w, dtype=np.float32)
    proj_b = np.asarray(proj_b, dtype=np.float32)

    nc = _get_program()
    in_maps = _make_in_maps(x, qkv_w, q_bias, v_bias, proj_w, proj_b)
    _cache["in_maps"] = in_maps

    res = run_bass_kernel_spmd(nc, in_maps, list(range(N_CORES)))
    out = np.empty((B, N, C), np.float32)
    for c in range(N_CORES):
        b, half = c // 2, c % 2
        out[b, half * NQ:(half + 1) * NQ, :] = res.results[c]["outT"].T
    return out
